# revision 7
# baseline (speedup 1.0000x reference)
"""Contrastive loss (video/audio) Trainium2 Bass kernel — v2 (transposed).

Full inputs: video [64,512,512] f32, audio [64,512,512] f32, mask [64,512] i32.
Data-parallel over batch: 8 cores x 8 batch elements; host sums the 8 partial
losses and divides by B.

Per-core design (v2): both modalities are loaded **d-transposed** via the DMA
xbar (``dma_start(transpose=True)``): dat[m] [128, (g:4, b:8, t:512)] bf16
with d = g*128 + p.  All d-contractions then run on the TensorEngine:

  s[j,t]  (row-dot-anchor)  = matmul(lhsT=sparse anchor col j, rhs=data)
  r[j,t]  (row sum-squares) = matmul(lhsT=sparse ones col j, rhs=squares)

Squares are elementwise TT(x,x) on DVE (2x bf16) / ACT Square, pairwise
*folded* (sq_g0+sq_g1, sq_g2+sq_g3) on DVE to halve the r matmul stream.
Both reductions accumulate junk-free into consolidated PSUM tiles [16,512]
(unit j: j<8 video b=j, j>=8 audio b=j-8) via [128,16] lhsT windows with only
column j nonzero (diagonal built with one stride-17 DVE copy per group).

Anchors: argmax(mask) on-chip ([16,512] pipeline, rows 8-15 replicate 0-7),
anchor rows gathered by indirect DMA (row b*512+idx_b of the *other*
modality), PE-transposed into per-group columns.  Tail: rsqrt(r), scale by
1/(TEMP*||anchor||), exp + row-sum, pos extracted via the argmax one-hot,
log/combine on [16,1], PE ones-reduce -> [1,1] partial loss.
"""

import numpy as np
from contextlib import ExitStack

import concourse.bass as bass
import concourse.tile as tile
from concourse import mybir
from concourse.bass_utils import run_bass_kernel_spmd

F32 = mybir.dt.float32
BF16 = mybir.dt.bfloat16
I32 = mybir.dt.int32
AF = mybir.ActivationFunctionType
OP = mybir.AluOpType
AX = mybir.AxisListType

B, T, D = 64, 512, 512
NCORES = 8
BL = B // NCORES          # 8 batch elements per core
P = 128                   # partitions
G = D // P                # 4 d-groups (d = g*128 + p)
NU = 2 * BL               # 16 units: j<8 video-b, j>=8 audio-b
TEMP = 0.07
BT = BL * T               # 4096 t-columns per (modality, group)
CW = 2 * T                # square/fold chunk width (2 batch elements)
HB = BT // 2              # DMA half (4 batch elements = 2048 rows)


def build_kernel(ctx: ExitStack, tc: tile.TileContext, video, audio, mask, out):
    nc = tc.nc

    persist = ctx.enter_context(tc.tile_pool(name="persist", bufs=1))
    sqp = ctx.enter_context(tc.tile_pool(name="sqp", bufs=6))
    psum_rs = ctx.enter_context(tc.tile_pool(name="psum_rs", bufs=1, space="PSUM"))
    psum_t = ctx.enter_context(tc.tile_pool(name="psum_t", bufs=2, space="PSUM"))

    src_flat = [video.rearrange("b t d -> (b t) d"),
                audio.rearrange("b t d -> (b t) d")]

    # ---------------- bulk transposed loads --------------------------------
    # dat[m] [128, (g,b,t)] bf16: col(g,b,t) = g*4096 + b*512 + t
    dat = [persist.tile([P, G * BT], BF16, tag=f"dat{m}", name=f"dat{m}")
           for m in range(2)]
    # 16 DMAs: (half h, g-pair, m, g) so fold pairs complete together
    for h in range(2):
        rows = slice(h * HB, (h + 1) * HB)
        for pair in range(2):
            for m in range(2):
                for g in (2 * pair, 2 * pair + 1):
                    nc.sync.dma_start(
                        out=dat[m][:, g * BT + h * HB:g * BT + (h + 1) * HB],
                        in_=src_flat[m][rows, g * P:(g + 1) * P],
                        transpose=True)

    # ---------------- mask -> idx / one-hot (rows 8-15 = rows 0-7) ---------
    mask16 = persist.tile([NU, T], I32, tag="mask16")
    nc.sync.dma_start(mask16[0:BL, :], mask[:, :])
    nc.sync.dma_start(mask16[BL:NU, :], mask[:, :])
    mask_f = persist.tile([NU, T], F32, tag="mask_f")
    nc.vector.tensor_copy(mask_f[:], mask16[:])
    iota_i = persist.tile([NU, T], I32, tag="iota_i")
    nc.gpsimd.iota(iota_i[:], pattern=[[1, T]], base=0, channel_multiplier=0)
    iota_f = persist.tile([NU, T], F32, tag="iota_f")
    nc.vector.tensor_copy(iota_f[:], iota_i[:])
    score = persist.tile([NU, T], F32, tag="score")
    nc.vector.scalar_tensor_tensor(
        out=score[:], in0=mask_f[:], scalar=1024.0, in1=iota_f[:],
        op0=OP.mult, op1=OP.subtract)
    maxs = persist.tile([NU, 1], F32, tag="maxs")
    nc.vector.reduce_max(maxs[:], score[:], axis=AX.X)
    onehot = persist.tile([NU, T], F32, tag="onehot")
    nc.vector.tensor_scalar(out=onehot[:], in0=score[:],
                            scalar1=maxs[:, :1], scalar2=None,
                            op0=OP.is_equal)
    # idx = sum(onehot * iota); flat row = b*512 + idx
    idx_f = persist.tile([NU, 1], F32, tag="idx_f")
    idx_scr = persist.tile([NU, T], F32, tag="idx_scr")
    nc.vector.scalar_tensor_tensor(
        out=idx_scr[:], in0=onehot[:], scalar=1.0, in1=iota_f[:],
        op0=OP.mult, op1=OP.mult, accum_out=idx_f[:])
    # bidx = (j mod 8) * T  built as j*T - (j>=8)*8*T (no partition slicing)
    ji = persist.tile([NU, 1], I32, tag="ji")
    nc.gpsimd.iota(ji[:], pattern=[[1, 1]], base=0, channel_multiplier=1)
    jf = persist.tile([NU, 1], F32, tag="jf")
    nc.vector.tensor_copy(jf[:], ji[:])
    hi8 = persist.tile([NU, 1], F32, tag="hi8")
    nc.vector.tensor_scalar(out=hi8[:], in0=jf[:], scalar1=float(BL),
                            scalar2=float(BL * T), op0=OP.is_ge,
                            op1=OP.mult)
    bidx_f = persist.tile([NU, 1], F32, tag="bidx_f")
    nc.vector.scalar_tensor_tensor(
        out=bidx_f[:], in0=jf[:], scalar=float(T), in1=hi8[:],
        op0=OP.mult, op1=OP.subtract)
    flat_f = persist.tile([NU, 1], F32, tag="flat_f")
    nc.vector.tensor_tensor(flat_f[:], idx_f[:], bidx_f[:], op=OP.add)
    flat_i = persist.tile([NU, 1], I32, tag="flat_i")
    nc.vector.tensor_copy(flat_i[:], flat_f[:])

    # ---------------- anchor gather + norms --------------------------------
    # rows 0-7: audio anchors (for video units), rows 8-15: video anchors
    anch = persist.tile([NU, D], BF16, tag="anch")
    nc.gpsimd.indirect_dma_start(
        out=anch[0:BL, :], out_offset=None, in_=src_flat[1],
        in_offset=bass.IndirectOffsetOnAxis(ap=flat_i[0:BL, :1], axis=0))
    nc.gpsimd.indirect_dma_start(
        out=anch[BL:NU, :], out_offset=None, in_=src_flat[0],
        in_offset=bass.IndirectOffsetOnAxis(ap=flat_i[BL:NU, :1], axis=0))
    r_anch = persist.tile([NU, 1], F32, tag="r_anch")
    ra_scr = persist.tile([NU, D], BF16, tag="ra_scr")
    nc.vector.scalar_tensor_tensor(
        out=ra_scr[:], in0=anch[:], scalar=1.0, in1=anch[:],
        op0=OP.mult, op1=OP.mult, accum_out=r_anch[:])
    # unit_scale = 1 / (TEMP * ||anchor||)
    sqr_anch = persist.tile([NU, 1], F32, tag="sqr_anch")
    nc.scalar.activation(sqr_anch[:], r_anch[:], AF.Sqrt, scale=TEMP * TEMP)
    unit_scale = persist.tile([NU, 1], F32, tag="unit_scale")
    nc.vector.reciprocal(unit_scale[:], sqr_anch[:])

    # ---------------- anchors transposed + sparse lhsT tiles ---------------
    eyei = persist.tile([NU, NU], I32, tag="eyei")
    nc.gpsimd.iota(eyei[:], pattern=[[1, NU]], base=0, channel_multiplier=-1)
    eyez = persist.tile([NU, NU], I32, tag="eyez")
    nc.vector.tensor_scalar(out=eyez[:], in0=eyei[:], scalar1=0,
                            scalar2=None, op0=OP.is_equal)
    eyef = persist.tile([NU, NU], BF16, tag="eyef")
    nc.vector.tensor_copy(eyef[:], eyez[:])
    # lhsT_s [128, (g, j, 16)]: window (g,j) = cols [g*256+j*16, +16), col j
    # nonzero = anchor_j[d(g)]  (diagonal at stride 17 within each g block)
    lhsT_s = persist.tile([P, G * NU * NU], BF16, tag="lhsT_s")
    nc.vector.memset(lhsT_s[:], 0.0)
    for g in range(G):
        tp = psum_t.tile([P, NU], BF16, tag="anch_tp")
        nc.tensor.transpose(out=tp[:], in_=anch[:, g * P:(g + 1) * P],
                            identity=eyef[:])
        base = g * NU * NU
        nc.vector.tensor_copy(
            lhsT_s[:, base:base + (NU - 1) * (NU + 1) + 1:NU + 1], tp[:])
    # lhsT_r [128, (j, 16)]: window j = cols [j*16, +16), col j = 1
    lhsT_r = persist.tile([P, NU * NU], BF16, tag="lhsT_r")
    nc.vector.memset(lhsT_r[:], 0.0)
    for j in range(NU):
        nc.vector.memset(lhsT_r[:, j * NU + j:j * NU + j + 1], 1.0)

    # ---------------- PSUM accumulators ------------------------------------
    s_psum = psum_rs.tile([NU, T], F32, tag="s_psum")
    r_psum = psum_rs.tile([NU, T], F32, tag="r_psum")

    # ---------------- main loop: squares, folds, matmuls -------------------
    folds = [[persist.tile([P, BT], BF16, tag=f"fold{m}{h}",
                           name=f"fold{m}{h}")
              for h in range(2)] for m in range(2)]
    sq_ctr = [0]
    mm_flags = {"s_first": True, "r_first": True, "s_n": 0, "r_n": 0}
    N_S = 2 * BL * G          # 64
    N_R = 2 * BL * 2          # 32

    def r_mm(j, rhs):
        nc.tensor.matmul(
            out=r_psum[:], lhsT=lhsT_r[:, j * NU:(j + 1) * NU], rhs=rhs,
            start=mm_flags["r_first"], stop=(mm_flags["r_n"] == N_R - 1))
        mm_flags["r_first"] = False
        mm_flags["r_n"] += 1

    def s_mm(j, g, rhs):
        base = g * NU * NU + j * NU
        nc.tensor.matmul(
            out=s_psum[:], lhsT=lhsT_s[:, base:base + NU], rhs=rhs,
            start=mm_flags["s_first"], stop=(mm_flags["s_n"] == N_S - 1))
        mm_flags["s_first"] = False
        mm_flags["s_n"] += 1

    for h in range(2):
        for pair in range(2):
            g0, g1 = 2 * pair, 2 * pair + 1
            for m in range(2):
                for cc in range(2):
                    c_abs = h * 2 + cc
                    w0 = c_abs * CW
                    sq_pair = []
                    for g in (g0, g1):
                        src = dat[m][:, g * BT + w0:g * BT + w0 + CW]
                        sq = sqp.tile([P, CW], BF16, tag=f"sq{g % 2}")
                        if sq_ctr[0] % 2 == 0:
                            nc.vector.tensor_tensor(sq[:], src, src,
                                                    op=OP.mult)
                        else:
                            nc.scalar.activation(sq[:], src, AF.Square)
                        sq_ctr[0] += 1
                        sq_pair.append(sq)
                    nc.vector.tensor_tensor(
                        folds[m][pair][:, w0:w0 + CW],
                        sq_pair[0][:], sq_pair[1][:], op=OP.add)
                    for bi in range(2):
                        b = 2 * c_abs + bi
                        j = m * BL + b
                        tw = slice(b * T, (b + 1) * T)
                        r_mm(j, folds[m][pair][:, tw])
                        s_mm(j, g0, dat[m][:, g0 * BT + b * T:
                                           g0 * BT + (b + 1) * T])
                        s_mm(j, g1, dat[m][:, g1 * BT + b * T:
                                           g1 * BT + (b + 1) * T])

    # ---------------- tail --------------------------------------------------
    srt = persist.tile([NU, T], F32, tag="srt")
    nc.scalar.activation(srt[:], r_psum[:], AF.Sqrt)
    rsr = persist.tile([NU, T], F32, tag="rsr")
    nc.vector.reciprocal(rsr[:], srt[:])
    sh = persist.tile([NU, T], F32, tag="sh")
    nc.vector.scalar_tensor_tensor(
        out=sh[:], in0=s_psum[:], scalar=unit_scale[:, :1], in1=rsr[:],
        op0=OP.mult, op1=OP.mult)
    exp_t = persist.tile([NU, T], BF16, tag="exp_t")
    sum_exp = persist.tile([NU, 1], F32, tag="sum_exp")
    nc.scalar.activation(exp_t[:], sh[:], AF.Exp, accum_out=sum_exp[:])
    pos = persist.tile([NU, 1], F32, tag="pos")
    pos_scr = persist.tile([NU, T], F32, tag="pos_scr")
    nc.vector.scalar_tensor_tensor(
        out=pos_scr[:], in0=sh[:], scalar=1.0, in1=onehot[:],
        op0=OP.mult, op1=OP.mult, accum_out=pos[:])
    epos = persist.tile([NU, 1], F32, tag="epos")
    nc.scalar.activation(epos[:], pos[:], AF.Exp)
    neg = persist.tile([NU, 1], F32, tag="neg")
    nc.vector.tensor_tensor(neg[:], sum_exp[:], epos[:], op=OP.subtract)
    lg = persist.tile([NU, 1], F32, tag="lg")
    nc.scalar.activation(lg[:], neg[:], AF.Ln)
    term = persist.tile([NU, 1], F32, tag="term")
    nc.vector.tensor_tensor(term[:], lg[:], pos[:], op=OP.subtract)
    nc.vector.tensor_scalar_mul(term[:], term[:], 0.5)
    ones16 = persist.tile([NU, 1], F32, tag="ones16")
    nc.vector.memset(ones16[:], 1.0)
    tot_ps = psum_t.tile([1, 1], F32, tag="tot_ps")
    nc.tensor.matmul(out=tot_ps[:], lhsT=ones16[:], rhs=term[:],
                     start=True, stop=True)
    tot = persist.tile([1, 1], F32, tag="tot")
    nc.vector.tensor_copy(tot[:], tot_ps[:])
    nc.sync.dma_start(out[:, :], tot[:])


# ---------------------------------------------------------------------------
# BIR legalization for this walrus build:
#  - it rejects instructions carrying more than one semaphore wait
#    ("Too many sync wait commands"): hoist extra waits onto single-wait
#    NoOp carriers on the same engine.
#  - the Tile tail's EVENT_SEMAPHORE_RANGE_CLEAR raw-ISA encoding mismatches
#    ("ISA wrong length"): replace with a sem-resetting Drain and drop the
#    trailing barrier that only fences the reset.
_LEGALIZE_N = [0]


def _legalize(nc):
    for fn in nc.m.functions:
        for bb in fn.blocks:
            new = []
            tail_trim = False
            for inst in bb.instructions:
                si = inst.sync_info
                if si is not None and si.on_wait and len(si.on_wait) > 1:
                    for w in list(si.on_wait[:-1]):
                        _LEGALIZE_N[0] += 1
                        new.append(mybir.InstNoOp(
                            name=f"I-lw{_LEGALIZE_N[0]}",
                            opcode="NoOp",
                            engine=inst.engine,
                            sync_info=mybir.SyncInfo(on_wait=[w],
                                                     on_update=[]),
                        ))
                    si.on_wait = [si.on_wait[-1]]
                if (isinstance(inst, mybir.InstISA)
                        and getattr(inst, "op_name", "") ==
                        "EVENT_SEMAPHORE_RANGE_CLEAR"):
                    ad = getattr(inst, "ant_dict", None) or {}
                    _LEGALIZE_N[0] += 1
                    new.append(mybir.InstDrain(
                        name=f"I-lc{_LEGALIZE_N[0]}",
                        opcode="Drain",
                        engine=inst.engine,
                        is_reset_sema=True,
                        reset_range_start=ad.get("range_first"),
                        reset_range_stop=ad.get("range_last"),
                    ))
                    tail_trim = True
                    continue
                if tail_trim and inst.opcode in ("EventSemaphore", "Drain"):
                    continue
                new.append(inst)
            bb.instructions[:] = new


_CACHE = {}


def _get_nc():
    if "nc" not in _CACHE:
        nc = bass.Bass("TRN2", target_bir_lowering=False, debug=False,
                       num_devices=NCORES)
        video = nc.dram_tensor("video", [BL, T, D], BF16,
                               kind="ExternalInput").ap()
        audio = nc.dram_tensor("audio", [BL, T, D], BF16,
                               kind="ExternalInput").ap()
        mask = nc.dram_tensor("mask", [BL, T], I32, kind="ExternalInput").ap()
        out = nc.dram_tensor("out", [1, 1], F32, kind="ExternalOutput").ap()
        with tile.TileContext(nc) as tc:
            with ExitStack() as ctx:
                build_kernel(ctx, tc, video, audio, mask, out)
        _legalize(nc)
        _CACHE["nc"] = nc
    return _CACHE["nc"]


def kernel(video, audio, mask, _want_results=False):
    import ml_dtypes
    video = np.ascontiguousarray(np.asarray(video).astype(ml_dtypes.bfloat16))
    audio = np.ascontiguousarray(np.asarray(audio).astype(ml_dtypes.bfloat16))
    mask = np.ascontiguousarray(np.asarray(mask, dtype=np.int32))
    nc = _get_nc()
    in_maps = []
    for i in range(NCORES):
        sl = slice(i * BL, (i + 1) * BL)
        in_maps.append({"video": video[sl], "audio": audio[sl],
                        "mask": mask[sl]})
    res = run_bass_kernel_spmd(nc, in_maps, list(range(NCORES)))
    parts = [res.results[i]["out"][0, 0] for i in range(NCORES)]
    loss = np.float32(np.sum(np.asarray(parts, dtype=np.float64)) / B)
    outarr = np.asarray([loss], dtype=np.float32)
    if _want_results:
        return outarr, res
    return outarr


USE_BF16 = True  # for test.py compatibility


# revision 8
# speedup vs baseline: 1.0027x; 1.0027x over previous
"""Contrastive loss (video/audio) Trainium2 Bass kernel — v2 (transposed).

Full inputs: video [64,512,512] f32, audio [64,512,512] f32, mask [64,512] i32.
Data-parallel over batch: 8 cores x 8 batch elements; host sums the 8 partial
losses and divides by B.

Per-core design (v2): both modalities are loaded **d-transposed** via the DMA
xbar (``dma_start(transpose=True)``): dat[m] [128, (g:4, b:8, t:512)] bf16
with d = g*128 + p.  All d-contractions then run on the TensorEngine:

  s[j,t]  (row-dot-anchor)  = matmul(lhsT=sparse anchor col j, rhs=data)
  r[j,t]  (row sum-squares) = matmul(lhsT=sparse ones col j, rhs=squares)

Squares are elementwise TT(x,x) on DVE (2x bf16) / ACT Square, pairwise
*folded* (sq_g0+sq_g1, sq_g2+sq_g3) on DVE to halve the r matmul stream.
Both reductions accumulate junk-free into consolidated PSUM tiles [16,512]
(unit j: j<8 video b=j, j>=8 audio b=j-8) via [128,16] lhsT windows with only
column j nonzero (diagonal built with one stride-17 DVE copy per group).

Anchors: argmax(mask) on-chip ([16,512] pipeline, rows 8-15 replicate 0-7),
anchor rows gathered by indirect DMA (row b*512+idx_b of the *other*
modality), PE-transposed into per-group columns.  Tail: rsqrt(r), scale by
1/(TEMP*||anchor||), exp + row-sum, pos extracted via the argmax one-hot,
log/combine on [16,1], PE ones-reduce -> [1,1] partial loss.
"""

import numpy as np
from contextlib import ExitStack

import concourse.bass as bass
import concourse.tile as tile
from concourse import mybir
from concourse.bass_utils import run_bass_kernel_spmd

F32 = mybir.dt.float32
BF16 = mybir.dt.bfloat16
I32 = mybir.dt.int32
AF = mybir.ActivationFunctionType
OP = mybir.AluOpType
AX = mybir.AxisListType

B, T, D = 64, 512, 512
NCORES = 8
BL = B // NCORES          # 8 batch elements per core
P = 128                   # partitions
G = D // P                # 4 d-groups (d = g*128 + p)
NU = 2 * BL               # 16 units: j<8 video-b, j>=8 audio-b
TEMP = 0.07
BT = BL * T               # 4096 t-columns per (modality, group)
CW = 2 * T                # square/fold chunk width (2 batch elements)
HB = BT // 2              # DMA half (4 batch elements = 2048 rows)


def build_kernel(ctx: ExitStack, tc: tile.TileContext, video, audio, mask, out):
    nc = tc.nc

    persist = ctx.enter_context(tc.tile_pool(name="persist", bufs=1))
    sqp = ctx.enter_context(tc.tile_pool(name="sqp", bufs=6))
    psum_rs = ctx.enter_context(tc.tile_pool(name="psum_rs", bufs=1, space="PSUM"))
    psum_t = ctx.enter_context(tc.tile_pool(name="psum_t", bufs=2, space="PSUM"))

    src_flat = [video.rearrange("b t d -> (b t) d"),
                audio.rearrange("b t d -> (b t) d")]

    # ---------------- bulk transposed loads --------------------------------
    # dat[m] [128, (g,b,t)] bf16: col(g,b,t) = g*4096 + b*512 + t
    dat = [persist.tile([P, G * BT], BF16, tag=f"dat{m}", name=f"dat{m}")
           for m in range(2)]
    # 4 full-width DMAs: (half h, m); out is [128, g:4, rows] (3D: extra dim
    # is logically part of the partition dim -> col d = g*128+p)
    for h in range(2):
        rows = slice(h * HB, (h + 1) * HB)
        for m in range(2):
            out3 = dat[m][:].rearrange("p (g bt) -> p g bt", g=G)[:, :, rows]
            eng = nc.sync if m == 0 else nc.scalar
            eng.dma_start(out=out3, in_=src_flat[m][rows, :], transpose=True)

    # ---------------- mask -> idx / one-hot (rows 8-15 = rows 0-7) ---------
    mask16 = persist.tile([NU, T], I32, tag="mask16")
    nc.sync.dma_start(mask16[0:BL, :], mask[:, :])
    nc.sync.dma_start(mask16[BL:NU, :], mask[:, :])
    mask_f = persist.tile([NU, T], F32, tag="mask_f")
    nc.vector.tensor_copy(mask_f[:], mask16[:])
    iota_i = persist.tile([NU, T], I32, tag="iota_i")
    nc.gpsimd.iota(iota_i[:], pattern=[[1, T]], base=0, channel_multiplier=0)
    iota_f = persist.tile([NU, T], F32, tag="iota_f")
    nc.vector.tensor_copy(iota_f[:], iota_i[:])
    score = persist.tile([NU, T], F32, tag="score")
    nc.vector.scalar_tensor_tensor(
        out=score[:], in0=mask_f[:], scalar=1024.0, in1=iota_f[:],
        op0=OP.mult, op1=OP.subtract)
    maxs = persist.tile([NU, 1], F32, tag="maxs")
    nc.vector.reduce_max(maxs[:], score[:], axis=AX.X)
    onehot = persist.tile([NU, T], F32, tag="onehot")
    nc.vector.tensor_scalar(out=onehot[:], in0=score[:],
                            scalar1=maxs[:, :1], scalar2=None,
                            op0=OP.is_equal)
    # idx = sum(onehot * iota); flat row = b*512 + idx
    idx_f = persist.tile([NU, 1], F32, tag="idx_f")
    idx_scr = persist.tile([NU, T], F32, tag="idx_scr")
    nc.vector.scalar_tensor_tensor(
        out=idx_scr[:], in0=onehot[:], scalar=1.0, in1=iota_f[:],
        op0=OP.mult, op1=OP.mult, accum_out=idx_f[:])
    # bidx = (j mod 8) * T  built as j*T - (j>=8)*8*T (no partition slicing)
    ji = persist.tile([NU, 1], I32, tag="ji")
    nc.gpsimd.iota(ji[:], pattern=[[1, 1]], base=0, channel_multiplier=1)
    jf = persist.tile([NU, 1], F32, tag="jf")
    nc.vector.tensor_copy(jf[:], ji[:])
    hi8 = persist.tile([NU, 1], F32, tag="hi8")
    nc.vector.tensor_scalar(out=hi8[:], in0=jf[:], scalar1=float(BL),
                            scalar2=float(BL * T), op0=OP.is_ge,
                            op1=OP.mult)
    bidx_f = persist.tile([NU, 1], F32, tag="bidx_f")
    nc.vector.scalar_tensor_tensor(
        out=bidx_f[:], in0=jf[:], scalar=float(T), in1=hi8[:],
        op0=OP.mult, op1=OP.subtract)
    flat_f = persist.tile([NU, 1], F32, tag="flat_f")
    nc.vector.tensor_tensor(flat_f[:], idx_f[:], bidx_f[:], op=OP.add)
    flat_i = persist.tile([NU, 1], I32, tag="flat_i")
    nc.vector.tensor_copy(flat_i[:], flat_f[:])

    # ---------------- anchor gather + norms --------------------------------
    # rows 0-7: audio anchors (for video units), rows 8-15: video anchors
    anch = persist.tile([NU, D], BF16, tag="anch")
    nc.gpsimd.indirect_dma_start(
        out=anch[0:BL, :], out_offset=None, in_=src_flat[1],
        in_offset=bass.IndirectOffsetOnAxis(ap=flat_i[0:BL, :1], axis=0))
    nc.gpsimd.indirect_dma_start(
        out=anch[BL:NU, :], out_offset=None, in_=src_flat[0],
        in_offset=bass.IndirectOffsetOnAxis(ap=flat_i[BL:NU, :1], axis=0))
    r_anch = persist.tile([NU, 1], F32, tag="r_anch")
    ra_scr = persist.tile([NU, D], BF16, tag="ra_scr")
    nc.vector.scalar_tensor_tensor(
        out=ra_scr[:], in0=anch[:], scalar=1.0, in1=anch[:],
        op0=OP.mult, op1=OP.mult, accum_out=r_anch[:])
    # unit_scale = 1 / (TEMP * ||anchor||)
    sqr_anch = persist.tile([NU, 1], F32, tag="sqr_anch")
    nc.scalar.activation(sqr_anch[:], r_anch[:], AF.Sqrt, scale=TEMP * TEMP)
    unit_scale = persist.tile([NU, 1], F32, tag="unit_scale")
    nc.vector.reciprocal(unit_scale[:], sqr_anch[:])

    # ---------------- anchors transposed + sparse lhsT tiles ---------------
    eyei = persist.tile([NU, NU], I32, tag="eyei")
    nc.gpsimd.iota(eyei[:], pattern=[[1, NU]], base=0, channel_multiplier=-1)
    eyez = persist.tile([NU, NU], I32, tag="eyez")
    nc.vector.tensor_scalar(out=eyez[:], in0=eyei[:], scalar1=0,
                            scalar2=None, op0=OP.is_equal)
    eyef = persist.tile([NU, NU], BF16, tag="eyef")
    nc.vector.tensor_copy(eyef[:], eyez[:])
    # lhsT_s [128, (g, j, 16)]: window (g,j) = cols [g*256+j*16, +16), col j
    # nonzero = anchor_j[d(g)]  (diagonal at stride 17 within each g block)
    lhsT_s = persist.tile([P, G * NU * NU], BF16, tag="lhsT_s")
    nc.vector.memset(lhsT_s[:], 0.0)
    for g in range(G):
        tp = psum_t.tile([P, NU], BF16, tag="anch_tp")
        nc.tensor.transpose(out=tp[:], in_=anch[:, g * P:(g + 1) * P],
                            identity=eyef[:])
        base = g * NU * NU
        nc.vector.tensor_copy(
            lhsT_s[:, base:base + (NU - 1) * (NU + 1) + 1:NU + 1], tp[:])
    # lhsT_r [128, (j, 16)]: window j = cols [j*16, +16), col j = 1
    lhsT_r = persist.tile([P, NU * NU], BF16, tag="lhsT_r")
    nc.vector.memset(lhsT_r[:], 0.0)
    for j in range(NU):
        nc.vector.memset(lhsT_r[:, j * NU + j:j * NU + j + 1], 1.0)

    # ---------------- PSUM accumulators ------------------------------------
    s_psum = psum_rs.tile([NU, T], F32, tag="s_psum")
    r_psum = psum_rs.tile([NU, T], F32, tag="r_psum")

    # ---------------- main loop: squares, folds, matmuls -------------------
    folds = [[persist.tile([P, BT], BF16, tag=f"fold{m}{h}",
                           name=f"fold{m}{h}")
              for h in range(2)] for m in range(2)]
    sq_ctr = [0]
    mm_flags = {"s_first": True, "r_first": True, "s_n": 0, "r_n": 0}
    N_S = 2 * BL * G          # 64
    N_R = 2 * BL * 2          # 32

    def r_mm(j, rhs):
        nc.tensor.matmul(
            out=r_psum[:], lhsT=lhsT_r[:, j * NU:(j + 1) * NU], rhs=rhs,
            start=mm_flags["r_first"], stop=(mm_flags["r_n"] == N_R - 1))
        mm_flags["r_first"] = False
        mm_flags["r_n"] += 1

    def s_mm(j, g, rhs):
        base = g * NU * NU + j * NU
        nc.tensor.matmul(
            out=s_psum[:], lhsT=lhsT_s[:, base:base + NU], rhs=rhs,
            start=mm_flags["s_first"], stop=(mm_flags["s_n"] == N_S - 1))
        mm_flags["s_first"] = False
        mm_flags["s_n"] += 1

    for h in range(2):
        for pair in range(2):
            g0, g1 = 2 * pair, 2 * pair + 1
            for m in range(2):
                for cc in range(2):
                    c_abs = h * 2 + cc
                    w0 = c_abs * CW
                    sq_pair = []
                    for g in (g0, g1):
                        src = dat[m][:, g * BT + w0:g * BT + w0 + CW]
                        sq = sqp.tile([P, CW], BF16, tag=f"sq{g % 2}")
                        if sq_ctr[0] % 2 == 0:
                            nc.vector.tensor_tensor(sq[:], src, src,
                                                    op=OP.mult)
                        else:
                            nc.scalar.activation(sq[:], src, AF.Square)
                        sq_ctr[0] += 1
                        sq_pair.append(sq)
                    nc.vector.tensor_tensor(
                        folds[m][pair][:, w0:w0 + CW],
                        sq_pair[0][:], sq_pair[1][:], op=OP.add)
                    for bi in range(2):
                        b = 2 * c_abs + bi
                        j = m * BL + b
                        tw = slice(b * T, (b + 1) * T)
                        r_mm(j, folds[m][pair][:, tw])
                        s_mm(j, g0, dat[m][:, g0 * BT + b * T:
                                           g0 * BT + (b + 1) * T])
                        s_mm(j, g1, dat[m][:, g1 * BT + b * T:
                                           g1 * BT + (b + 1) * T])

    # ---------------- tail --------------------------------------------------
    srt = persist.tile([NU, T], F32, tag="srt")
    nc.scalar.activation(srt[:], r_psum[:], AF.Sqrt)
    rsr = persist.tile([NU, T], F32, tag="rsr")
    nc.vector.reciprocal(rsr[:], srt[:])
    sh = persist.tile([NU, T], F32, tag="sh")
    nc.vector.scalar_tensor_tensor(
        out=sh[:], in0=s_psum[:], scalar=unit_scale[:, :1], in1=rsr[:],
        op0=OP.mult, op1=OP.mult)
    exp_t = persist.tile([NU, T], BF16, tag="exp_t")
    sum_exp = persist.tile([NU, 1], F32, tag="sum_exp")
    nc.scalar.activation(exp_t[:], sh[:], AF.Exp, accum_out=sum_exp[:])
    pos = persist.tile([NU, 1], F32, tag="pos")
    pos_scr = persist.tile([NU, T], F32, tag="pos_scr")
    nc.vector.scalar_tensor_tensor(
        out=pos_scr[:], in0=sh[:], scalar=1.0, in1=onehot[:],
        op0=OP.mult, op1=OP.mult, accum_out=pos[:])
    epos = persist.tile([NU, 1], F32, tag="epos")
    nc.scalar.activation(epos[:], pos[:], AF.Exp)
    neg = persist.tile([NU, 1], F32, tag="neg")
    nc.vector.tensor_tensor(neg[:], sum_exp[:], epos[:], op=OP.subtract)
    lg = persist.tile([NU, 1], F32, tag="lg")
    nc.scalar.activation(lg[:], neg[:], AF.Ln)
    term = persist.tile([NU, 1], F32, tag="term")
    nc.vector.tensor_tensor(term[:], lg[:], pos[:], op=OP.subtract)
    nc.vector.tensor_scalar_mul(term[:], term[:], 0.5)
    ones16 = persist.tile([NU, 1], F32, tag="ones16")
    nc.vector.memset(ones16[:], 1.0)
    tot_ps = psum_t.tile([1, 1], F32, tag="tot_ps")
    nc.tensor.matmul(out=tot_ps[:], lhsT=ones16[:], rhs=term[:],
                     start=True, stop=True)
    tot = persist.tile([1, 1], F32, tag="tot")
    nc.vector.tensor_copy(tot[:], tot_ps[:])
    nc.sync.dma_start(out[:, :], tot[:])


# ---------------------------------------------------------------------------
# BIR legalization for this walrus build:
#  - it rejects instructions carrying more than one semaphore wait
#    ("Too many sync wait commands"): hoist extra waits onto single-wait
#    NoOp carriers on the same engine.
#  - the Tile tail's EVENT_SEMAPHORE_RANGE_CLEAR raw-ISA encoding mismatches
#    ("ISA wrong length"): replace with a sem-resetting Drain and drop the
#    trailing barrier that only fences the reset.
_LEGALIZE_N = [0]


def _legalize(nc):
    for fn in nc.m.functions:
        for bb in fn.blocks:
            new = []
            tail_trim = False
            for inst in bb.instructions:
                si = inst.sync_info
                if si is not None and si.on_wait and len(si.on_wait) > 1:
                    for w in list(si.on_wait[:-1]):
                        _LEGALIZE_N[0] += 1
                        new.append(mybir.InstNoOp(
                            name=f"I-lw{_LEGALIZE_N[0]}",
                            opcode="NoOp",
                            engine=inst.engine,
                            sync_info=mybir.SyncInfo(on_wait=[w],
                                                     on_update=[]),
                        ))
                    si.on_wait = [si.on_wait[-1]]
                if (isinstance(inst, mybir.InstISA)
                        and getattr(inst, "op_name", "") ==
                        "EVENT_SEMAPHORE_RANGE_CLEAR"):
                    ad = getattr(inst, "ant_dict", None) or {}
                    _LEGALIZE_N[0] += 1
                    new.append(mybir.InstDrain(
                        name=f"I-lc{_LEGALIZE_N[0]}",
                        opcode="Drain",
                        engine=inst.engine,
                        is_reset_sema=True,
                        reset_range_start=ad.get("range_first"),
                        reset_range_stop=ad.get("range_last"),
                    ))
                    tail_trim = True
                    continue
                if tail_trim and inst.opcode in ("EventSemaphore", "Drain"):
                    continue
                new.append(inst)
            bb.instructions[:] = new


_CACHE = {}


def _get_nc():
    if "nc" not in _CACHE:
        nc = bass.Bass("TRN2", target_bir_lowering=False, debug=False,
                       num_devices=NCORES)
        video = nc.dram_tensor("video", [BL, T, D], BF16,
                               kind="ExternalInput").ap()
        audio = nc.dram_tensor("audio", [BL, T, D], BF16,
                               kind="ExternalInput").ap()
        mask = nc.dram_tensor("mask", [BL, T], I32, kind="ExternalInput").ap()
        out = nc.dram_tensor("out", [1, 1], F32, kind="ExternalOutput").ap()
        with tile.TileContext(nc) as tc:
            with ExitStack() as ctx:
                build_kernel(ctx, tc, video, audio, mask, out)
        _legalize(nc)
        _CACHE["nc"] = nc
    return _CACHE["nc"]


def kernel(video, audio, mask, _want_results=False):
    import ml_dtypes
    video = np.ascontiguousarray(np.asarray(video).astype(ml_dtypes.bfloat16))
    audio = np.ascontiguousarray(np.asarray(audio).astype(ml_dtypes.bfloat16))
    mask = np.ascontiguousarray(np.asarray(mask, dtype=np.int32))
    nc = _get_nc()
    in_maps = []
    for i in range(NCORES):
        sl = slice(i * BL, (i + 1) * BL)
        in_maps.append({"video": video[sl], "audio": audio[sl],
                        "mask": mask[sl]})
    res = run_bass_kernel_spmd(nc, in_maps, list(range(NCORES)))
    parts = [res.results[i]["out"][0, 0] for i in range(NCORES)]
    loss = np.float32(np.sum(np.asarray(parts, dtype=np.float64)) / B)
    outarr = np.asarray([loss], dtype=np.float32)
    if _want_results:
        return outarr, res
    return outarr


USE_BF16 = True  # for test.py compatibility


# revision 11
# speedup vs baseline: 1.0578x; 1.0549x over previous
"""Contrastive loss (video/audio) Trainium2 Bass kernel — v2 (transposed).

Full inputs: video [64,512,512] f32, audio [64,512,512] f32, mask [64,512] i32.
Data-parallel over batch: 8 cores x 8 batch elements; host sums the 8 partial
losses and divides by B.

Per-core design (v2): both modalities are loaded **d-transposed** via the DMA
xbar (``dma_start(transpose=True)``): dat[m] [128, (g:4, b:8, t:512)] bf16
with d = g*128 + p.  All d-contractions then run on the TensorEngine:

  s[j,t]  (row-dot-anchor)  = matmul(lhsT=sparse anchor col j, rhs=data)
  r[j,t]  (row sum-squares) = matmul(lhsT=sparse ones col j, rhs=squares)

Squares are elementwise TT(x,x) on DVE (2x bf16) / ACT Square, pairwise
*folded* (sq_g0+sq_g1, sq_g2+sq_g3) on DVE to halve the r matmul stream.
Both reductions accumulate junk-free into consolidated PSUM tiles [16,512]
(unit j: j<8 video b=j, j>=8 audio b=j-8) via [128,16] lhsT windows with only
column j nonzero (diagonal built with one stride-17 DVE copy per group).

Anchors: argmax(mask) on-chip ([16,512] pipeline, rows 8-15 replicate 0-7),
anchor rows gathered by indirect DMA (row b*512+idx_b of the *other*
modality), PE-transposed into per-group columns.  Tail: rsqrt(r), scale by
1/(TEMP*||anchor||), exp + row-sum, pos extracted via the argmax one-hot,
log/combine on [16,1], PE ones-reduce -> [1,1] partial loss.
"""

import numpy as np
from contextlib import ExitStack

import concourse.bass as bass
import concourse.tile as tile
from concourse import mybir
from concourse.bass_utils import run_bass_kernel_spmd

F32 = mybir.dt.float32
BF16 = mybir.dt.bfloat16
I32 = mybir.dt.int32
AF = mybir.ActivationFunctionType
OP = mybir.AluOpType
AX = mybir.AxisListType

B, T, D = 64, 512, 512
NCORES = 8
BL = B // NCORES          # 8 batch elements per core
P = 128                   # partitions
G = D // P                # 4 d-groups (d = g*128 + p)
NU = 2 * BL               # 16 units: j<8 video-b, j>=8 audio-b
TEMP = 0.07
BT = BL * T               # 4096 t-columns per (modality, group)
CW = 2 * T                # square/fold chunk width (2 batch elements)
HB = BT // 2              # DMA half (4 batch elements = 2048 rows)


def build_kernel(ctx: ExitStack, tc: tile.TileContext, video, audio, mask, out):
    nc = tc.nc

    persist = ctx.enter_context(tc.tile_pool(name="persist", bufs=1))
    sqp = ctx.enter_context(tc.tile_pool(name="sqp", bufs=6))
    psum_rs = ctx.enter_context(tc.tile_pool(name="psum_rs", bufs=1, space="PSUM"))
    psum_t = ctx.enter_context(tc.tile_pool(name="psum_t", bufs=2, space="PSUM"))

    src_flat = [video.rearrange("b t d -> (b t) d"),
                audio.rearrange("b t d -> (b t) d")]

    # ---------------- bulk transposed loads --------------------------------
    # dat[m] [128, (h, g, b', t)] bf16: col(g, b, t) =
    #   (b//4)*G*HB + g*HB + (b%4)*T + t  -- each DMA half h writes the
    # contiguous region [h*G*HB, (h+1)*G*HB)  (HB = 2048 = 4 b's of t)
    dat = [persist.tile([P, G * BT], BF16, tag=f"dat{m}", name=f"dat{m}")
           for m in range(2)]

    def dcol(g, b):
        return (b // 4) * G * HB + g * HB + (b % 4) * T

    # 4 full-width DMAs: (half h, m); out is [128, g:4, rows] (3D: extra
    # dims are logically part of the partition dim -> src col d = g*128+p)
    for h in range(2):
        rows = slice(h * HB, (h + 1) * HB)
        for m in range(2):
            out3 = dat[m][:, h * G * HB:(h + 1) * G * HB] \
                .rearrange("p (g bt) -> p g bt", g=G)
            nc.sync.dma_start(out=out3, in_=src_flat[m][rows, :],
                              transpose=True)

    # ---------------- mask -> idx / one-hot (rows 8-15 = rows 0-7) ---------
    mask16 = persist.tile([NU, T], I32, tag="mask16")
    nc.sync.dma_start(mask16[0:BL, :], mask[:, :])
    nc.sync.dma_start(mask16[BL:NU, :], mask[:, :])
    mask_f = persist.tile([NU, T], F32, tag="mask_f")
    nc.vector.tensor_copy(mask_f[:], mask16[:])
    iota_i = persist.tile([NU, T], I32, tag="iota_i")
    nc.gpsimd.iota(iota_i[:], pattern=[[1, T]], base=0, channel_multiplier=0)
    iota_f = persist.tile([NU, T], F32, tag="iota_f")
    nc.vector.tensor_copy(iota_f[:], iota_i[:])
    score = persist.tile([NU, T], F32, tag="score")
    nc.vector.scalar_tensor_tensor(
        out=score[:], in0=mask_f[:], scalar=1024.0, in1=iota_f[:],
        op0=OP.mult, op1=OP.subtract)
    maxs = persist.tile([NU, 1], F32, tag="maxs")
    nc.vector.reduce_max(maxs[:], score[:], axis=AX.X)
    onehot = persist.tile([NU, T], F32, tag="onehot")
    nc.vector.tensor_scalar(out=onehot[:], in0=score[:],
                            scalar1=maxs[:, :1], scalar2=None,
                            op0=OP.is_equal)
    # idx = sum(onehot * iota); flat row = b*512 + idx
    idx_f = persist.tile([NU, 1], F32, tag="idx_f")
    idx_scr = persist.tile([NU, T], F32, tag="idx_scr")
    nc.vector.scalar_tensor_tensor(
        out=idx_scr[:], in0=onehot[:], scalar=1.0, in1=iota_f[:],
        op0=OP.mult, op1=OP.mult, accum_out=idx_f[:])
    # bidx = (j mod 8) * T  built as j*T - (j>=8)*8*T (no partition slicing)
    ji = persist.tile([NU, 1], I32, tag="ji")
    nc.gpsimd.iota(ji[:], pattern=[[1, 1]], base=0, channel_multiplier=1)
    jf = persist.tile([NU, 1], F32, tag="jf")
    nc.vector.tensor_copy(jf[:], ji[:])
    hi8 = persist.tile([NU, 1], F32, tag="hi8")
    nc.vector.tensor_scalar(out=hi8[:], in0=jf[:], scalar1=float(BL),
                            scalar2=float(BL * T), op0=OP.is_ge,
                            op1=OP.mult)
    bidx_f = persist.tile([NU, 1], F32, tag="bidx_f")
    nc.vector.scalar_tensor_tensor(
        out=bidx_f[:], in0=jf[:], scalar=float(T), in1=hi8[:],
        op0=OP.mult, op1=OP.subtract)
    flat_f = persist.tile([NU, 1], F32, tag="flat_f")
    nc.vector.tensor_tensor(flat_f[:], idx_f[:], bidx_f[:], op=OP.add)
    flat_i = persist.tile([NU, 1], I32, tag="flat_i")
    nc.vector.tensor_copy(flat_i[:], flat_f[:])

    # ---------------- anchor gather + norms --------------------------------
    # rows 0-7: audio anchors (for video units), rows 8-15: video anchors
    anch = persist.tile([NU, D], BF16, tag="anch")
    nc.gpsimd.indirect_dma_start(
        out=anch[0:BL, :], out_offset=None, in_=src_flat[1],
        in_offset=bass.IndirectOffsetOnAxis(ap=flat_i[0:BL, :1], axis=0))
    nc.gpsimd.indirect_dma_start(
        out=anch[BL:NU, :], out_offset=None, in_=src_flat[0],
        in_offset=bass.IndirectOffsetOnAxis(ap=flat_i[BL:NU, :1], axis=0))
    r_anch = persist.tile([NU, 1], F32, tag="r_anch")
    ra_scr = persist.tile([NU, D], BF16, tag="ra_scr")
    nc.vector.scalar_tensor_tensor(
        out=ra_scr[:], in0=anch[:], scalar=1.0, in1=anch[:],
        op0=OP.mult, op1=OP.mult, accum_out=r_anch[:])
    # unit_scale = 1 / (TEMP * ||anchor||)
    sqr_anch = persist.tile([NU, 1], F32, tag="sqr_anch")
    nc.scalar.activation(sqr_anch[:], r_anch[:], AF.Sqrt, scale=TEMP * TEMP)
    unit_scale = persist.tile([NU, 1], F32, tag="unit_scale")
    nc.vector.reciprocal(unit_scale[:], sqr_anch[:])

    # ---------------- anchors transposed + sparse lhsT tiles ---------------
    eyei = persist.tile([NU, NU], I32, tag="eyei")
    nc.gpsimd.iota(eyei[:], pattern=[[1, NU]], base=0, channel_multiplier=-1)
    eyez = persist.tile([NU, NU], I32, tag="eyez")
    nc.vector.tensor_scalar(out=eyez[:], in0=eyei[:], scalar1=0,
                            scalar2=None, op0=OP.is_equal)
    eyef = persist.tile([NU, NU], BF16, tag="eyef")
    nc.vector.tensor_copy(eyef[:], eyez[:])
    # lhsT_s [128, (g, j, 16)]: window (g,j) = cols [g*256+j*16, +16), col j
    # nonzero = anchor_j[d(g)]  (diagonal at stride 17 within each g block)
    lhsT_s = persist.tile([P, G * NU * NU], BF16, tag="lhsT_s")
    nc.vector.memset(lhsT_s[:], 0.0)
    for g in range(G):
        tp = psum_t.tile([P, NU], BF16, tag="anch_tp")
        nc.tensor.transpose(out=tp[:], in_=anch[:, g * P:(g + 1) * P],
                            identity=eyef[:])
        base = g * NU * NU
        nc.vector.tensor_copy(
            lhsT_s[:, base:base + (NU - 1) * (NU + 1) + 1:NU + 1], tp[:])
    # lhsT_r [128, (j, 16)]: window j = cols [j*16, +16), col j = 1
    lhsT_r = persist.tile([P, NU * NU], BF16, tag="lhsT_r")
    nc.vector.memset(lhsT_r[:], 0.0)
    for j in range(NU):
        nc.vector.memset(lhsT_r[:, j * NU + j:j * NU + j + 1], 1.0)

    # ---------------- PSUM accumulators ------------------------------------
    s_psum = psum_rs.tile([NU, T], F32, tag="s_psum")
    r_psum = psum_rs.tile([NU, T], F32, tag="r_psum")

    # ---------------- main loop: squares, folds, matmuls -------------------
    folds = [[persist.tile([P, BT], BF16, tag=f"fold{m}{h}",
                           name=f"fold{m}{h}")
              for h in range(2)] for m in range(2)]
    sq_ctr = [0]
    mm_flags = {"s_first": True, "r_first": True, "s_n": 0, "r_n": 0}
    N_S = 2 * BL * G          # 64
    N_R = 2 * BL * 2          # 32

    def r_mm(j, rhs):
        nc.tensor.matmul(
            out=r_psum[:], lhsT=lhsT_r[:, j * NU:(j + 1) * NU], rhs=rhs,
            start=mm_flags["r_first"], stop=(mm_flags["r_n"] == N_R - 1))
        mm_flags["r_first"] = False
        mm_flags["r_n"] += 1

    def s_mm(j, g, rhs):
        base = g * NU * NU + j * NU
        nc.tensor.matmul(
            out=s_psum[:], lhsT=lhsT_s[:, base:base + NU], rhs=rhs,
            start=mm_flags["s_first"], stop=(mm_flags["s_n"] == N_S - 1))
        mm_flags["s_first"] = False
        mm_flags["s_n"] += 1

    for h in range(2):
        for pair in range(2):
            g0, g1 = 2 * pair, 2 * pair + 1
            for m in range(2):
                for cc in range(2):
                    c_abs = h * 2 + cc
                    w0 = c_abs * CW
                    sq_pair = []
                    for g in (g0, g1):
                        src0 = dcol(g, 2 * c_abs)
                        src = dat[m][:, src0:src0 + CW]
                        sq = sqp.tile([P, CW], BF16, tag=f"sq{g % 2}")
                        if sq_ctr[0] % 2 == 0:
                            nc.vector.tensor_tensor(sq[:], src, src,
                                                    op=OP.mult)
                        else:
                            nc.scalar.activation(sq[:], src, AF.Square)
                        sq_ctr[0] += 1
                        sq_pair.append(sq)
                    nc.vector.tensor_tensor(
                        folds[m][pair][:, w0:w0 + CW],
                        sq_pair[0][:], sq_pair[1][:], op=OP.add)
                    for bi in range(2):
                        b = 2 * c_abs + bi
                        j = m * BL + b
                        tw = slice(b * T, (b + 1) * T)
                        r_mm(j, folds[m][pair][:, tw])
                        s_mm(j, g0, dat[m][:, dcol(g0, b):dcol(g0, b) + T])
                        s_mm(j, g1, dat[m][:, dcol(g1, b):dcol(g1, b) + T])

    # ---------------- tail --------------------------------------------------
    srt = persist.tile([NU, T], F32, tag="srt")
    nc.scalar.activation(srt[:], r_psum[:], AF.Sqrt)
    rsr = persist.tile([NU, T], F32, tag="rsr")
    nc.vector.reciprocal(rsr[:], srt[:])
    sh = persist.tile([NU, T], F32, tag="sh")
    nc.vector.scalar_tensor_tensor(
        out=sh[:], in0=s_psum[:], scalar=unit_scale[:, :1], in1=rsr[:],
        op0=OP.mult, op1=OP.mult)
    exp_t = persist.tile([NU, T], BF16, tag="exp_t")
    sum_exp = persist.tile([NU, 1], F32, tag="sum_exp")
    nc.scalar.activation(exp_t[:], sh[:], AF.Exp, accum_out=sum_exp[:])
    pos = persist.tile([NU, 1], F32, tag="pos")
    pos_scr = persist.tile([NU, T], F32, tag="pos_scr")
    nc.vector.scalar_tensor_tensor(
        out=pos_scr[:], in0=sh[:], scalar=1.0, in1=onehot[:],
        op0=OP.mult, op1=OP.mult, accum_out=pos[:])
    epos = persist.tile([NU, 1], F32, tag="epos")
    nc.scalar.activation(epos[:], pos[:], AF.Exp)
    neg = persist.tile([NU, 1], F32, tag="neg")
    nc.vector.tensor_tensor(neg[:], sum_exp[:], epos[:], op=OP.subtract)
    lg = persist.tile([NU, 1], F32, tag="lg")
    nc.scalar.activation(lg[:], neg[:], AF.Ln)
    term = persist.tile([NU, 1], F32, tag="term")
    nc.vector.tensor_tensor(term[:], lg[:], pos[:], op=OP.subtract)
    nc.vector.tensor_scalar_mul(term[:], term[:], 0.5)
    ones16 = persist.tile([NU, 1], F32, tag="ones16")
    nc.vector.memset(ones16[:], 1.0)
    tot_ps = psum_t.tile([1, 1], F32, tag="tot_ps")
    nc.tensor.matmul(out=tot_ps[:], lhsT=ones16[:], rhs=term[:],
                     start=True, stop=True)
    tot = persist.tile([1, 1], F32, tag="tot")
    nc.vector.tensor_copy(tot[:], tot_ps[:])
    nc.sync.dma_start(out[:, :], tot[:])


# ---------------------------------------------------------------------------
# BIR legalization for this walrus build:
#  - it rejects instructions carrying more than one semaphore wait
#    ("Too many sync wait commands"): hoist extra waits onto single-wait
#    NoOp carriers on the same engine.
#  - the Tile tail's EVENT_SEMAPHORE_RANGE_CLEAR raw-ISA encoding mismatches
#    ("ISA wrong length"): replace with a sem-resetting Drain and drop the
#    trailing barrier that only fences the reset.
_LEGALIZE_N = [0]


def _legalize(nc):
    for fn in nc.m.functions:
        for bb in fn.blocks:
            new = []
            tail_trim = False
            for inst in bb.instructions:
                si = inst.sync_info
                if si is not None and si.on_wait and len(si.on_wait) > 1:
                    for w in list(si.on_wait[:-1]):
                        _LEGALIZE_N[0] += 1
                        new.append(mybir.InstNoOp(
                            name=f"I-lw{_LEGALIZE_N[0]}",
                            opcode="NoOp",
                            engine=inst.engine,
                            sync_info=mybir.SyncInfo(on_wait=[w],
                                                     on_update=[]),
                        ))
                    si.on_wait = [si.on_wait[-1]]
                if (isinstance(inst, mybir.InstISA)
                        and getattr(inst, "op_name", "") ==
                        "EVENT_SEMAPHORE_RANGE_CLEAR"):
                    ad = getattr(inst, "ant_dict", None) or {}
                    _LEGALIZE_N[0] += 1
                    new.append(mybir.InstDrain(
                        name=f"I-lc{_LEGALIZE_N[0]}",
                        opcode="Drain",
                        engine=inst.engine,
                        is_reset_sema=True,
                        reset_range_start=ad.get("range_first"),
                        reset_range_stop=ad.get("range_last"),
                    ))
                    tail_trim = True
                    continue
                if tail_trim and inst.opcode in ("EventSemaphore", "Drain"):
                    continue
                new.append(inst)
            bb.instructions[:] = new


_CACHE = {}


def _get_nc():
    if "nc" not in _CACHE:
        nc = bass.Bass("TRN2", target_bir_lowering=False, debug=False,
                       num_devices=NCORES)
        video = nc.dram_tensor("video", [BL, T, D], BF16,
                               kind="ExternalInput").ap()
        audio = nc.dram_tensor("audio", [BL, T, D], BF16,
                               kind="ExternalInput").ap()
        mask = nc.dram_tensor("mask", [BL, T], I32, kind="ExternalInput").ap()
        out = nc.dram_tensor("out", [1, 1], F32, kind="ExternalOutput").ap()
        with tile.TileContext(nc) as tc:
            with ExitStack() as ctx:
                build_kernel(ctx, tc, video, audio, mask, out)
        _legalize(nc)
        _CACHE["nc"] = nc
    return _CACHE["nc"]


def kernel(video, audio, mask, _want_results=False):
    import ml_dtypes
    video = np.ascontiguousarray(np.asarray(video).astype(ml_dtypes.bfloat16))
    audio = np.ascontiguousarray(np.asarray(audio).astype(ml_dtypes.bfloat16))
    mask = np.ascontiguousarray(np.asarray(mask, dtype=np.int32))
    nc = _get_nc()
    in_maps = []
    for i in range(NCORES):
        sl = slice(i * BL, (i + 1) * BL)
        in_maps.append({"video": video[sl], "audio": audio[sl],
                        "mask": mask[sl]})
    res = run_bass_kernel_spmd(nc, in_maps, list(range(NCORES)))
    parts = [res.results[i]["out"][0, 0] for i in range(NCORES)]
    loss = np.float32(np.sum(np.asarray(parts, dtype=np.float64)) / B)
    outarr = np.asarray([loss], dtype=np.float32)
    if _want_results:
        return outarr, res
    return outarr


USE_BF16 = True  # for test.py compatibility
